# revision 1
# baseline (speedup 1.0000x reference)
"""Causal self-attention, head-tensor-parallel across 8 TRN2 NeuronCores.

Problem: x[2,2048,1024] -> qkv = x@W_attn+b_attn -> 16-head causal attention
(head dim 64) -> y@W_proj+b_proj.

Sharding: heads are tensor-parallel. Core c owns heads 2c and 2c+1:
  - W_attn column slices for its q/k/v features (384 cols), W_proj row slice
    (128 rows). Every core reads all of x (transposed+bf16 on host).
  - Each core emits a full [4096,1024] fp32 partial of the output projection;
    the host sums the 8 partials and adds b_proj.

On-core dataflow (all matmuls bf16 in / fp32 PSUM accum):
  1. qkv^T[384,4096] = W_slice^T @ x^T     (features on partitions)
  2. V     = PE-transpose of v^T, augmented with a ones column (row sums of
             P fall out of the AV matmul as column 64 -> softmax denominator)
  3. S^T[k,q] = k^T.T @ q^T  per head, causally block-skipped; the two heads
     run row-group-packed (contraction K=64 at partitions 0-63 / 64-127).
     exp via ScalarE with scale=1/8 (the 1/sqrt(D) factor), bf16 out = P^T.
     Diagonal blocks get a triu mask multiply after exp (no max subtraction:
     |S| < 10 for this distribution, exp stays tiny vs fp32 range).
  4. y_aug[q,65] = P^T.T @ V_aug accumulated over k tiles; normalize by
     reciprocal of column 64 (per-partition scalar).
  5. y^T via PE transpose, then out[tok,1024] = y^T.T @ W_proj_slice.
"""

import numpy as np
import ml_dtypes

import concourse.bacc as bacc
import concourse.bass as bass
import concourse.mybir as mybir
import concourse.tile as tile
from concourse.bass_utils import run_bass_kernel_spmd
from concourse.masks import make_identity

BF16 = mybir.dt.bfloat16
FP16 = mybir.dt.float16
FP32 = mybir.dt.float32

B, T, C, H = 2, 2048, 1024, 16
D = C // H            # 64
N_CORES = 8
HPC = H // N_CORES    # heads per core = 2
TOK = B * T           # 4096
P = 128               # partitions / tile edge
KT = T // P           # 16 k/q tiles per batch element
NQ = 1024             # S^T / exp chunk width (2 PSUM banks)
XC = 1024             # x^T token chunk for streaming

bf16 = ml_dtypes.bfloat16


def _pt_offsets():
    """Column offsets of each k-tile's ragged [k, q] strip in the P^T store."""
    offs, total = [], 0
    for t in range(KT):
        offs.append(total)
        total += T - P * t
    return offs, total


PT_OFF, PT_COLS = _pt_offsets()  # PT_COLS = 17408


def build_nc():
    # Bacc (not raw Bass): its lowering legalizes TRN2's one-wait-per-
    # instruction constraint by splitting multi-waits into EventSemaphores.
    nc = bacc.Bacc("TRN2", target_bir_lowering=False, debug=False)

    # x^T pre-packed on host into contiguous per-chunk DMA layout:
    # [chunk, p, ktile, tok] with element = x[chunk*XC+tok, ktile*128+p]
    xT = nc.dram_tensor(
        "xT", [TOK // XC, P, C // P, XC], BF16, kind="ExternalInput"
    ).ap()
    w_qkv = nc.dram_tensor("w_qkv", [C, 3 * P], BF16, kind="ExternalInput").ap()
    b_qkv = nc.dram_tensor("b_qkv", [3 * P], FP32, kind="ExternalInput").ap()
    w_p = nc.dram_tensor("w_p", [P, C], BF16, kind="ExternalInput").ap()
    out_p = nc.dram_tensor("out_p", [TOK, C], FP16, kind="ExternalOutput").ap()

    with TileOwner(nc) as tc:
        _emit(nc, tc, xT, w_qkv, b_qkv, w_p, out_p)
    nc.compile()
    return nc


class TileOwner:
    """Thin wrapper so build_nc reads top-down; just a TileContext."""

    def __init__(self, nc):
        self._tc = tile.TileContext(nc)

    def __enter__(self):
        return self._tc.__enter__()

    def __exit__(self, *a):
        return self._tc.__exit__(*a)


def _emit(nc, tc, xT, w_qkv, b_qkv, w_p, out_p):
    from contextlib import ExitStack

    ctx = ExitStack()
    with ctx:
        consts = ctx.enter_context(tc.tile_pool(name="consts", bufs=1))
        persist = ctx.enter_context(tc.tile_pool(name="persist", bufs=1))

        # ---- constants ----
        # w_qkv + bias ride the sync HWDGE queue in k-tile slices interleaved
        # with chunk 0's x slices (emitted inside emit_qkv0) so the first
        # matmul's operands land within ~2us instead of waiting out the full
        # 2MB chunk DMA. w_p / identities aren't needed until the tail.
        w_qkv_sb = consts.tile([P, C // P, 3 * P], BF16)  # [p, ktile, feat]
        bias_sb = consts.tile([P, 3], FP32)  # col m: bias of feature m*128+p
        nc.sync.dma_start(out=bias_sb, in_=b_qkv.rearrange("(m p) -> p m", p=P))
        w_p_sb = consts.tile([P, C], BF16)
        nc.gpsimd.dma_start(out=w_p_sb, in_=w_p)
        ident = consts.tile([P, P], BF16)
        make_identity(nc, ident)

        # ---- persistent activations ----
        qT = persist.tile([P, TOK], BF16)   # rows: head A dims 0-63, head B 64-127
        kTt = persist.tile([P, TOK], BF16)
        vT = persist.tile([P, TOK], BF16)
        qkvT = [qT, kTt, vT]
        # V augmented with ones column, per (b, head): [k-in-tile, ktile, D+1]
        v_aug = [
            [persist.tile([P, KT, D + 1], BF16, name=f"v_aug_{b}_{h}") for h in range(HPC)]
            for b in range(B)
        ]
        yn_all = persist.tile([P, TOK], BF16)   # normalized y, [q, feat128] blocks
        yT_sb = persist.tile([P, TOK], BF16)    # y^T, feat on partitions
        # ragged P^T store, one per head (reused across b; serializes b0/b1)
        pt_sb = [persist.tile([P, PT_COLS], BF16, name=f"pt_{h}") for h in range(HPC)]

        # ---- pipeline ----
        # Emission order sets Tile's scheduling priority. S(b0) is emitted
        # between the two QKV halves so ScalarE starts the exp stream ~40us
        # earlier, with QKV(2,3) acting as PE gap-filler while exp paces the
        # S matmuls. PSUM banks: qkv(2) + vt(2) + s(4) = 8 in the overlap
        # window; later o/yt globals(3) + av(4) = 7.
        # SBUF-side tail pools (allocated below xT so xT pops cleanly)
        osb = ctx.enter_context(tc.tile_pool(name="o_sb", bufs=4))
        # 8 bufs/tag: b1's AV evicts all buffer in SBUF during the exp(b1)
        # window; their transposes+tails drain after exp ends.
        ytr_pool = ctx.enter_context(tc.tile_pool(name="ytr_pool", bufs=8))
        recips = ctx.enter_context(tc.tile_pool(name="recips", bufs=8))
        xp = tc.alloc_tile_pool(name="xT_pool", bufs=2)
        # chunk 0 runs k-tile-outer with all 6 accumulators live so the PE
        # starts as soon as the first (w, x) k-slice pair lands; its pool is
        # released before the steady-state pools go down.
        qps0 = tc.alloc_tile_pool(name="qkv0_ps", bufs=1, space="PSUM")

        def emit_qkv0():
            # DMA the first chunk in 2-ktile pieces, weights interleaved, so
            # the kt=0 matmuls are runnable after ~0.7MB instead of ~2.8MB.
            x_sb = xp.tile([P, C // P, XC], BF16, name="x_sb")
            for g in range(C // P // 2):
                k0, k1 = 2 * g, 2 * g + 2
                for kt in range(k0, k1):
                    nc.sync.dma_start(
                        out=w_qkv_sb[:, kt, :], in_=w_qkv[kt * P : (kt + 1) * P, :]
                    )
                nc.sync.dma_start(out=x_sb[:, k0:k1, :], in_=xT[0][:, k0:k1, :])
            accs = [qps0.tile([P, 512], FP32, name=f"q0_acc{i}") for i in range(6)]
            for kt in range(C // P):
                for mi in range(3):
                    for j in range(2):
                        nc.tensor.matmul(
                            accs[mi * 2 + j],
                            w_qkv_sb[:, kt, mi * P : (mi + 1) * P],
                            x_sb[:, kt, j * 512 : (j + 1) * 512],
                            start=(kt == 0),
                            stop=(kt == C // P - 1),
                        )
            for mi in range(3):
                for j in range(2):
                    nc.vector.tensor_scalar_add(
                        out=qkvT[mi][:, j * 512 : (j + 1) * 512],
                        in0=accs[mi * 2 + j],
                        scalar1=bias_sb[:, mi : mi + 1],
                    )

        def emit_qkv(nch):
            x_sb = xp.tile([P, C // P, XC], BF16, name="x_sb")
            nc.sync.dma_start(out=x_sb, in_=xT[nch])
            for mi in range(3):
                for j in range(XC // 512):
                    ps = qps.tile([P, 512], FP32, name="qkv_acc")
                    for kt in range(C // P):
                        nc.tensor.matmul(
                            ps,
                            w_qkv_sb[:, kt, mi * P : (mi + 1) * P],
                            x_sb[:, kt, j * 512 : (j + 1) * 512],
                            start=(kt == 0),
                            stop=(kt == C // P - 1),
                        )
                    nc.vector.tensor_scalar_add(
                        out=qkvT[mi][
                            :, nch * XC + j * 512 : nch * XC + (j + 1) * 512
                        ],
                        in0=ps,
                        scalar1=bias_sb[:, mi : mi + 1],
                    )

        def emit_v(b, vtp):
            for h in range(HPC):
                nc.vector.memset(v_aug[b][h][:, :, D : D + 1], 1.0)
            for kt in range(KT):
                tok0 = b * T + kt * P
                ps_t = vtp.tile([P, P], BF16, name="vt_t")
                nc.tensor.transpose(ps_t, vT[:, tok0 : tok0 + P], ident)
                for h in range(HPC):
                    nc.vector.tensor_copy(
                        out=v_aug[b][h][:, kt, 0:D],
                        in_=ps_t[:, h * D : (h + 1) * D],
                    )

        def emit_s(b, sps, wlo=0, whi=PT_COLS // NQ, emitted_mask=None):
            # S^T / exp over the PACKED column space of the P^T store: the
            # causal strips are contiguous, so exp runs in uniform
            # [128, 1024] windows (17408 = 17*1024) instead of ragged
            # per-k-tile chunks -- fewer, fuller ScalarE instructions.
            # NOTE: Tile orders reads only against writes already emitted, so
            # a window may only be emitted after the qkv chunks covering its
            # q-columns (window 0 is the only chunk-0-pure one for b=0).
            if emitted_mask is None:
                emitted_mask = set()
            for w in range(wlo, whi):
                w0, w1 = w * NQ, (w + 1) * NQ
                ps_s = [sps.tile([P, NQ], FP32, name=f"s_acc_{h}") for h in range(HPC)]
                for kt in range(KT):
                    a = max(w0, PT_OFF[kt])
                    bnd = min(w1, PT_OFF[kt] + (T - P * kt))
                    if a >= bnd:
                        continue
                    ktok = b * T + kt * P
                    # split at PSUM bank (512) boundaries within the window
                    c = a
                    while c < bnd:
                        nxt = min(bnd, w0 + ((c - w0) // 512 + 1) * 512)
                        q0 = kt * P + (c - PT_OFF[kt])
                        for h in range(HPC):
                            rows = slice(h * D, (h + 1) * D)
                            nc.tensor.matmul(
                                ps_s[h][:, c - w0 : nxt - w0],
                                kTt[rows, ktok : ktok + P],
                                qT[rows, b * T + q0 : b * T + q0 + nxt - c],
                                start=True,
                                stop=True,
                            )
                        c = nxt
                for h in range(HPC):
                    nc.scalar.activation(
                        out=pt_sb[h][:, w0:w1],
                        in_=ps_s[h],
                        func=mybir.ActivationFunctionType.Exp,
                        scale=1.0 / np.sqrt(D),
                    )
                # causal masks for diagonal blocks fully covered so far
                for kt in range(KT):
                    if kt in emitted_mask or PT_OFF[kt] + P > w1:
                        continue
                    emitted_mask.add(kt)
                    for h in range(HPC):
                        nc.gpsimd.affine_select(
                            out=pt_sb[h][:, PT_OFF[kt] : PT_OFF[kt] + P],
                            in_=pt_sb[h][:, PT_OFF[kt] : PT_OFF[kt] + P],
                            pattern=[[1, P]],
                            compare_op=mybir.AluOpType.is_ge,
                            fill=0.0,
                            base=0,
                            channel_multiplier=-1,
                        )

        def _copy(eng, out, in_):
            # ScalarE copies are ACTIVATE-Copy; DVE copies are TensorCopy.
            if eng == "scalar":
                nc.scalar.copy(out, in_)
            else:
                nc.vector.tensor_copy(out=out, in_=in_)

        def emit_tail(gq, o_engines=("vector", "vector"), yt_engine="vector"):
            """y^T for q-tile gq, its projection chunk, evict + store."""
            ps_t2 = tps2.tile([P, P], BF16, name="yt_t")
            nc.tensor.transpose(ps_t2, yn_all[:, gq * P : (gq + 1) * P], ident)
            _copy(yt_engine, yT_sb[:, gq * P : (gq + 1) * P], ps_t2)
            o_sb = osb.tile([P, C], FP16, name="o_stage")
            for fj in range(C // 512):
                ps_o = ops.tile([P, 512], FP32, name="o_acc")
                nc.tensor.matmul(
                    ps_o,
                    yT_sb[:, gq * P : (gq + 1) * P],
                    w_p_sb[:, fj * 512 : (fj + 1) * 512],
                    start=True,
                    stop=True,
                )
                _copy(o_engines[fj], o_sb[:, fj * 512 : (fj + 1) * 512], ps_o)
            nc.sync.dma_start(out=out_p[gq * P : (gq + 1) * P, :], in_=o_sb)

        QQ = 512  # AV accumulator width (1 PSUM bank per head)

        def emit_av_mm(b, avp, ytr_engines):
            # AV in y^T orientation: V_aug stationary, P^T moving ->
            # y^T_aug[65, 512] accumulated over kt in PSUM, per 512-q chunk,
            # then evicted to bf16 SBUF staging (engine chosen per phase so
            # the evict never competes with the exp stream).
            ytrs_all = []
            for qc in range(T // QQ):
                q0, q1 = qc * QQ, (qc + 1) * QQ
                kmax = q1 // P - 1
                ps_ya = [
                    avp.tile([D + 1, QQ], FP32, name=f"yta_{h}")
                    for h in range(HPC)
                ]
                for h in range(HPC):
                    for kt in range(kmax + 1):
                        sub0 = max(q0, kt * P)
                        col0 = PT_OFF[kt] + sub0 - kt * P
                        nc.tensor.matmul(
                            ps_ya[h][:, sub0 - q0 : QQ],
                            v_aug[b][h][:, kt, :],
                            pt_sb[h][:, col0 : col0 + q1 - sub0],
                            start=(kt == 0),
                            stop=(kt == kmax),
                        )
                ytrs = []
                for h in range(HPC):
                    ytr = ytr_pool.tile([D + 1, QQ], BF16, name=f"ytr_{h}")
                    _copy(ytr_engines[h], ytr, ps_ya[h])
                    ytrs.append(ytr)
                ytrs_all.append(ytrs)
            return ytrs_all

        def emit_av_norm_tail(b, ytrs_all, avt, do_tails=True, alternate=False):
            # alternate=True flips the whole per-q-tile chain (normalize +
            # evicts) between DVE and ScalarE so the two engines' in-order
            # FIFOs pipeline adjacent q-tiles instead of one FIFO carrying
            # every step of every chain.
            for qc in range(T // QQ):
                ytrs = ytrs_all[qc]
                for qt in range(QQ // P):
                    gq = b * KT + qc * (QQ // P) + qt
                    eng = "scalar" if (alternate and qt % 2) else "vector"
                    for h in range(HPC):
                        ps_t = avt.tile([P, D + 1], BF16, name="av_t")
                        nc.tensor.transpose(
                            ps_t,
                            ytrs[h][:, qt * P : (qt + 1) * P],
                            ident[0 : D + 1, 0 : D + 1],
                        )
                        rc = recips.tile([P, 1], FP32, name=f"rc_{h}")
                        nc.vector.reciprocal(rc, ps_t[:, D : D + 1])
                        yn_slice = yn_all[:, gq * P + h * D : gq * P + (h + 1) * D]
                        if eng == "scalar":
                            nc.scalar.mul(yn_slice, ps_t[:, 0:D], rc)
                        else:
                            nc.vector.tensor_scalar_mul(
                                out=yn_slice, in0=ps_t[:, 0:D], scalar1=rc
                            )
                    if do_tails:
                        emit_tail(gq, o_engines=(eng, eng), yt_engine=eng)

        # b=0 tokens live in x chunks 0..T//XC-1
        emit_qkv0()
        qps0.release()
        # s_ps_0 sits at the bottom of the PSUM stack so qkv/vt (released
        # earlier) free their banks for the AV pools while S(b0) is live.
        # The pool is REUSED for S(b1): b1's window tiles WAR-chase b0's
        # drain, so exp(b1) starts while AV(b0) is still consuming pt.
        sps0 = tc.alloc_tile_pool(name="s_ps_0", bufs=1, space="PSUM")
        qps = tc.alloc_tile_pool(name="qkv_ps", bufs=2, space="PSUM")
        # S(b0) window 0 only needs chunk 0, so the exp stream starts ~12us
        # in; the remaining windows follow qkv(1) (their q-columns span
        # chunk 1 -- Tile cannot order reads against later-emitted writes).
        mask0 = set()
        emit_s(0, sps0, 0, 1, mask0)
        emit_qkv(1)
        with tc.tile_pool(name="vt_ps", bufs=2, space="PSUM") as vtp:
            emit_v(0, vtp)
            emit_s(0, sps0, 1, PT_COLS // NQ, mask0)
            for nch in range(T // XC, TOK // XC):
                emit_qkv(nch)
            emit_v(1, vtp)
        qps.release()
        xp.release()
        with tc.tile_pool(name="av_ps_0", bufs=1, space="PSUM") as avp, \
             tc.tile_pool(name="avt_ps_0", bufs=2, space="PSUM") as avt:
            # ytr evicts split scalar/vector: ScalarE idles between the two
            # exp streams here.
            y0 = emit_av_mm(0, avp, ("scalar", "vector"))
            emit_av_norm_tail(0, y0, avt, do_tails=False)
        with tc.tile_pool(name="o_ps", bufs=1, space="PSUM") as ops, \
             tc.tile_pool(name="yt_ps", bufs=1, space="PSUM") as tps2, \
             tc.tile_pool(name="av_ps_1", bufs=1, space="PSUM") as avp1:
            emit_s(1, sps0)
            # AV(b1) matmuls + b0's projection tails fill PE gaps while
            # S(b1) paces ScalarE (which is why all evicts here pin to DVE).
            y1 = emit_av_mm(1, avp1, ("vector", "vector"))
            for gq in range(KT):
                emit_tail(gq)
        sps0.release()
        # the final norm+tail burst gets the full PSUM budget: deep pools so
        # transposes / projections / evicts pipeline instead of trickling.
        with tc.tile_pool(name="o_ps2", bufs=3, space="PSUM") as ops, \
             tc.tile_pool(name="yt_ps2", bufs=2, space="PSUM") as tps2, \
             tc.tile_pool(name="avt_ps_1", bufs=3, space="PSUM") as avt:
            # exp is done: ScalarE takes alternate q-tile chains.
            emit_av_norm_tail(1, y1, avt, alternate=True)


def shard_inputs(x, W_attn, b_attn, W_proj, b_proj):
    x = np.asarray(x, np.float32)
    W_attn = np.asarray(W_attn, np.float32)
    b_attn = np.asarray(b_attn, np.float32)
    W_proj = np.asarray(W_proj, np.float32)

    # [chunk, p, ktile, tok]: contiguous per-chunk DMA source for x^T
    xT = np.ascontiguousarray(
        x.reshape(TOK // XC, XC, C // P, P).transpose(0, 3, 2, 1)
    ).astype(bf16)
    in_maps = []
    for c in range(N_CORES):
        fs = slice(P * c, P * (c + 1))
        w_slice = np.ascontiguousarray(
            np.concatenate(
                [W_attn[:, 0 * C + P * c : 0 * C + P * (c + 1)],
                 W_attn[:, 1 * C + P * c : 1 * C + P * (c + 1)],
                 W_attn[:, 2 * C + P * c : 2 * C + P * (c + 1)]],
                axis=1,
            )
        ).astype(bf16)
        b_slice = np.ascontiguousarray(
            np.concatenate([b_attn[0 * C + P * c : 0 * C + P * (c + 1)],
                            b_attn[1 * C + P * c : 1 * C + P * (c + 1)],
                            b_attn[2 * C + P * c : 2 * C + P * (c + 1)]])
        ).astype(np.float32)
        wp_slice = np.ascontiguousarray(W_proj[fs, :]).astype(bf16)
        in_maps.append(
            {"xT": xT, "w_qkv": w_slice, "b_qkv": b_slice, "w_p": wp_slice}
        )
    return in_maps


def kernel(x, W_attn, b_attn, W_proj, b_proj, _trace=False):
    in_maps = shard_inputs(x, W_attn, b_attn, W_proj, b_proj)
    nc = build_nc()
    res = run_bass_kernel_spmd(nc, in_maps, list(range(N_CORES)), trace=_trace)
    acc = np.zeros((TOK, C), np.float64)
    for r in res.results:
        acc += r["out_p"].astype(np.float64)
    out = acc.astype(np.float32) + np.asarray(b_proj, np.float32)[None, :]
    if _trace:
        kernel.last_results = res
    return out.reshape(B, T, C)



# revision 15
# speedup vs baseline: 1.0107x; 1.0107x over previous
"""Causal self-attention, head-tensor-parallel across 8 TRN2 NeuronCores.

Problem: x[2,2048,1024] -> qkv = x@W_attn+b_attn -> 16-head causal attention
(head dim 64) -> y@W_proj+b_proj.

Sharding: heads are tensor-parallel. Core c owns heads 2c and 2c+1:
  - W_attn column slices for its q/k/v features (384 cols), W_proj row slice
    (128 rows). Every core reads all of x (transposed+bf16 on host).
  - Each core emits a full [4096,1024] fp32 partial of the output projection;
    the host sums the 8 partials and adds b_proj.

On-core dataflow (all matmuls bf16 in / fp32 PSUM accum):
  1. qkv^T[384,4096] = W_slice^T @ x^T     (features on partitions)
  2. V     = PE-transpose of v^T, augmented with a ones column (row sums of
             P fall out of the AV matmul as column 64 -> softmax denominator)
  3. S^T[k,q] = k^T.T @ q^T  per head, causally block-skipped; the two heads
     run row-group-packed (contraction K=64 at partitions 0-63 / 64-127).
     exp via ScalarE with scale=1/8 (the 1/sqrt(D) factor), bf16 out = P^T.
     Diagonal blocks get a triu mask multiply after exp (no max subtraction:
     |S| < 10 for this distribution, exp stays tiny vs fp32 range).
  4. y_aug[q,65] = P^T.T @ V_aug accumulated over k tiles; normalize by
     reciprocal of column 64 (per-partition scalar).
  5. y^T via PE transpose, then out[tok,1024] = y^T.T @ W_proj_slice.
"""

import numpy as np
import ml_dtypes

import concourse.bacc as bacc
import concourse.bass as bass
import concourse.mybir as mybir
import concourse.tile as tile
from concourse.bass_utils import run_bass_kernel_spmd
from concourse.masks import make_identity

BF16 = mybir.dt.bfloat16
FP16 = mybir.dt.float16
FP32 = mybir.dt.float32

B, T, C, H = 2, 2048, 1024, 16
D = C // H            # 64
N_CORES = 8
HPC = H // N_CORES    # heads per core = 2
TOK = B * T           # 4096
P = 128               # partitions / tile edge
KT = T // P           # 16 k/q tiles per batch element
NQ = 1024             # S^T / exp chunk width (2 PSUM banks)
XC = 1024             # x^T token chunk for streaming

bf16 = ml_dtypes.bfloat16


def _pt_offsets():
    """Column offsets of each k-tile's ragged [k, q] strip in the P^T store."""
    offs, total = [], 0
    for t in range(KT):
        offs.append(total)
        total += T - P * t
    return offs, total


PT_OFF, PT_COLS = _pt_offsets()  # PT_COLS = 17408


def build_nc():
    # Bacc (not raw Bass): its lowering legalizes TRN2's one-wait-per-
    # instruction constraint by splitting multi-waits into EventSemaphores.
    nc = bacc.Bacc("TRN2", target_bir_lowering=False, debug=False)

    # x^T pre-packed on host into contiguous per-chunk DMA layout:
    # [chunk, p, ktile, tok] with element = x[chunk*XC+tok, ktile*128+p]
    xT = nc.dram_tensor(
        "xT", [TOK // XC, P, C // P, XC], BF16, kind="ExternalInput"
    ).ap()
    w_qkv = nc.dram_tensor("w_qkv", [C, 3 * P], BF16, kind="ExternalInput").ap()
    b_qkv = nc.dram_tensor("b_qkv", [P, 3], FP32, kind="ExternalInput").ap()
    w_p = nc.dram_tensor("w_p", [P, C], BF16, kind="ExternalInput").ap()
    out_p = nc.dram_tensor("out_p", [TOK, C], FP16, kind="ExternalOutput").ap()

    with TileOwner(nc) as tc:
        _emit(nc, tc, xT, w_qkv, b_qkv, w_p, out_p)
    nc.compile()
    return nc


class TileOwner:
    """Thin wrapper so build_nc reads top-down; just a TileContext."""

    def __init__(self, nc):
        self._tc = tile.TileContext(nc)

    def __enter__(self):
        return self._tc.__enter__()

    def __exit__(self, *a):
        return self._tc.__exit__(*a)


def _emit(nc, tc, xT, w_qkv, b_qkv, w_p, out_p):
    from contextlib import ExitStack

    ctx = ExitStack()
    with ctx:
        consts = ctx.enter_context(tc.tile_pool(name="consts", bufs=1))
        persist = ctx.enter_context(tc.tile_pool(name="persist", bufs=1))

        # ---- constants ----
        # DMA queue plan: w_qkv k-tile slices ride the sync HWDGE queue while
        # x chunk-0 pieces stream on the gpsimd queue in parallel, so the
        # first matmul's operands land ~10us in (preamble-limited). bias is
        # host-prepacked to [128,3] (the rearrange gather emitted 384 4-byte
        # descriptors that clogged the queue for ~10us) and deferred behind
        # the w slices. w_p / identities aren't needed until later.
        w_qkv_sb = consts.tile([P, C // P, 3 * P], BF16)  # [p, ktile, feat]
        bias_sb = consts.tile([P, 3], FP32)  # col m: bias of feature m*128+p
        w_p_sb = consts.tile([P, C], BF16)
        ident = consts.tile([P, P], BF16)

        # ---- persistent activations ----
        qT = persist.tile([P, TOK], BF16)   # rows: head A dims 0-63, head B 64-127
        kTt = persist.tile([P, TOK], BF16)
        vT = persist.tile([P, TOK], BF16)
        qkvT = [qT, kTt, vT]
        # V augmented with ones column, per (b, head): [k-in-tile, ktile, D+1]
        v_aug = [
            [persist.tile([P, KT, D + 1], BF16, name=f"v_aug_{b}_{h}") for h in range(HPC)]
            for b in range(B)
        ]
        yn_all = persist.tile([P, TOK], BF16)   # normalized y, [q, feat128] blocks
        yT_sb = persist.tile([P, TOK], BF16)    # y^T, feat on partitions
        # ragged P^T store, one per head (reused across b; serializes b0/b1)
        pt_sb = [persist.tile([P, PT_COLS], BF16, name=f"pt_{h}") for h in range(HPC)]

        # ---- pipeline ----
        # Emission order sets Tile's scheduling priority. S(b0) is emitted
        # between the two QKV halves so ScalarE starts the exp stream ~40us
        # earlier, with QKV(2,3) acting as PE gap-filler while exp paces the
        # S matmuls. PSUM banks: qkv(2) + vt(2) + s(4) = 8 in the overlap
        # window; later o/yt globals(3) + av(4) = 7.
        # SBUF-side tail pools (allocated below xT so xT pops cleanly)
        osb = ctx.enter_context(tc.tile_pool(name="o_sb", bufs=4))
        # 8 bufs/tag: b1's AV evicts all buffer in SBUF during the exp(b1)
        # window; their transposes+tails drain after exp ends.
        ytr_pool = ctx.enter_context(tc.tile_pool(name="ytr_pool", bufs=8))
        recips = ctx.enter_context(tc.tile_pool(name="recips", bufs=8))
        xp = tc.alloc_tile_pool(name="xT_pool", bufs=2)
        # chunk 0 runs k-tile-outer with all 6 accumulators live so the PE
        # starts as soon as the first (w, x) k-slice pair lands; its pool is
        # released before the steady-state pools go down.
        qps0 = tc.alloc_tile_pool(name="qkv0_ps", bufs=1, space="PSUM")

        def emit_qkv0():
            # x chunk 0 in 1-ktile pieces on the gpsimd queue, w slices on
            # sync in parallel: the kt=0 matmuls are runnable after ~0.35MB.
            x_sb = xp.tile([P, C // P, XC], BF16, name="x_sb")
            for kt in range(C // P):
                nc.sync.dma_start(
                    out=w_qkv_sb[:, kt, :], in_=w_qkv[kt * P : (kt + 1) * P, :]
                )
                nc.gpsimd.dma_start(out=x_sb[:, kt : kt + 1, :], in_=xT[0][:, kt : kt + 1, :])
            nc.sync.dma_start(out=bias_sb, in_=b_qkv)
            make_identity(nc, ident)
            accs = [qps0.tile([P, 512], FP32, name=f"q0_acc{i}") for i in range(6)]
            for kt in range(C // P):
                for mi in range(3):
                    for j in range(2):
                        nc.tensor.matmul(
                            accs[mi * 2 + j],
                            w_qkv_sb[:, kt, mi * P : (mi + 1) * P],
                            x_sb[:, kt, j * 512 : (j + 1) * 512],
                            start=(kt == 0),
                            stop=(kt == C // P - 1),
                        )
            for mi in range(3):
                for j in range(2):
                    nc.vector.tensor_scalar_add(
                        out=qkvT[mi][:, j * 512 : (j + 1) * 512],
                        in0=accs[mi * 2 + j],
                        scalar1=bias_sb[:, mi : mi + 1],
                    )

        def emit_qkv(nch, q="sync"):
            x_sb = xp.tile([P, C // P, XC], BF16, name="x_sb")
            getattr(nc, q).dma_start(out=x_sb, in_=xT[nch])
            for mi in range(3):
                for j in range(XC // 512):
                    ps = qps.tile([P, 512], FP32, name="qkv_acc")
                    for kt in range(C // P):
                        nc.tensor.matmul(
                            ps,
                            w_qkv_sb[:, kt, mi * P : (mi + 1) * P],
                            x_sb[:, kt, j * 512 : (j + 1) * 512],
                            start=(kt == 0),
                            stop=(kt == C // P - 1),
                        )
                    nc.vector.tensor_scalar_add(
                        out=qkvT[mi][
                            :, nch * XC + j * 512 : nch * XC + (j + 1) * 512
                        ],
                        in0=ps,
                        scalar1=bias_sb[:, mi : mi + 1],
                    )

        def emit_v(b, vtp):
            for h in range(HPC):
                nc.vector.memset(v_aug[b][h][:, :, D : D + 1], 1.0)
            for kt in range(KT):
                tok0 = b * T + kt * P
                ps_t = vtp.tile([P, P], BF16, name="vt_t")
                nc.tensor.transpose(ps_t, vT[:, tok0 : tok0 + P], ident)
                for h in range(HPC):
                    nc.vector.tensor_copy(
                        out=v_aug[b][h][:, kt, 0:D],
                        in_=ps_t[:, h * D : (h + 1) * D],
                    )

        def emit_s(b, sps, wlo=0, whi=None, emitted_mask=None, nq=NQ):
            NQ_ = nq
            if whi is None:
                whi = PT_COLS // NQ_
            # S^T / exp over the PACKED column space of the P^T store: the
            # causal strips are contiguous, so exp runs in uniform
            # [128, 1024] windows (17408 = 17*1024) instead of ragged
            # per-k-tile chunks -- fewer, fuller ScalarE instructions.
            # NOTE: Tile orders reads only against writes already emitted, so
            # a window may only be emitted after the qkv chunks covering its
            # q-columns (window 0 is the only chunk-0-pure one for b=0).
            if emitted_mask is None:
                emitted_mask = set()
            for w in range(wlo, whi):
                w0, w1 = w * NQ_, (w + 1) * NQ_
                ps_s = [sps.tile([P, NQ_], FP32, name=f"s_acc_{h}") for h in range(HPC)]
                for kt in range(KT):
                    a = max(w0, PT_OFF[kt])
                    bnd = min(w1, PT_OFF[kt] + (T - P * kt))
                    if a >= bnd:
                        continue
                    ktok = b * T + kt * P
                    # split at PSUM bank (512) boundaries within the window
                    c = a
                    while c < bnd:
                        nxt = min(bnd, w0 + ((c - w0) // 512 + 1) * 512)
                        q0 = kt * P + (c - PT_OFF[kt])
                        for h in range(HPC):
                            rows = slice(h * D, (h + 1) * D)
                            nc.tensor.matmul(
                                ps_s[h][:, c - w0 : nxt - w0],
                                kTt[rows, ktok : ktok + P],
                                qT[rows, b * T + q0 : b * T + q0 + nxt - c],
                                start=True,
                                stop=True,
                            )
                        c = nxt
                for h in range(HPC):
                    nc.scalar.activation(
                        out=pt_sb[h][:, w0:w1],
                        in_=ps_s[h],
                        func=mybir.ActivationFunctionType.Exp,
                        scale=1.0 / np.sqrt(D),
                    )
                # causal masks for diagonal blocks fully covered so far
                for kt in range(KT):
                    if kt in emitted_mask or PT_OFF[kt] + P > w1:
                        continue
                    emitted_mask.add(kt)
                    for h in range(HPC):
                        nc.gpsimd.affine_select(
                            out=pt_sb[h][:, PT_OFF[kt] : PT_OFF[kt] + P],
                            in_=pt_sb[h][:, PT_OFF[kt] : PT_OFF[kt] + P],
                            pattern=[[1, P]],
                            compare_op=mybir.AluOpType.is_ge,
                            fill=0.0,
                            base=0,
                            channel_multiplier=-1,
                        )

        def _copy(eng, out, in_):
            # ScalarE copies are ACTIVATE-Copy; DVE/Pool copies are TensorCopy.
            if eng == "scalar":
                nc.scalar.copy(out, in_)
            elif eng == "gpsimd":
                nc.gpsimd.tensor_copy(out=out, in_=in_)
            else:
                nc.vector.tensor_copy(out=out, in_=in_)

        def emit_tail(gq, o_engines=("vector", "vector"), yt_engine="vector"):
            """y^T for q-tile gq, its projection chunk, evict + store."""
            ps_t2 = tps2.tile([P, P], BF16, name="yt_t")
            nc.tensor.transpose(ps_t2, yn_all[:, gq * P : (gq + 1) * P], ident)
            _copy(yt_engine, yT_sb[:, gq * P : (gq + 1) * P], ps_t2)
            o_sb = osb.tile([P, C], FP16, name="o_stage")
            for fj in range(C // 512):
                ps_o = ops.tile([P, 512], FP32, name="o_acc")
                nc.tensor.matmul(
                    ps_o,
                    yT_sb[:, gq * P : (gq + 1) * P],
                    w_p_sb[:, fj * 512 : (fj + 1) * 512],
                    start=True,
                    stop=True,
                )
                _copy(o_engines[fj], o_sb[:, fj * 512 : (fj + 1) * 512], ps_o)
            nc.sync.dma_start(out=out_p[gq * P : (gq + 1) * P, :], in_=o_sb)

        QQ = 512  # AV accumulator width (1 PSUM bank per head)

        def emit_av_qc(b, qc, avp, ytr_engines):
            # AV in y^T orientation: V_aug stationary, P^T moving ->
            # y^T_aug[65, 512] accumulated over kt in PSUM, per 512-q chunk,
            # then evicted to bf16 SBUF staging (engine chosen per phase so
            # the evict never competes with the exp stream).
            q0, q1 = qc * QQ, (qc + 1) * QQ
            kmax = q1 // P - 1
            ps_ya = [
                avp.tile([D + 1, QQ], FP32, name=f"yta_{h}")
                for h in range(HPC)
            ]
            for h in range(HPC):
                for kt in range(kmax + 1):
                    sub0 = max(q0, kt * P)
                    col0 = PT_OFF[kt] + sub0 - kt * P
                    nc.tensor.matmul(
                        ps_ya[h][:, sub0 - q0 : QQ],
                        v_aug[b][h][:, kt, :],
                        pt_sb[h][:, col0 : col0 + q1 - sub0],
                        start=(kt == 0),
                        stop=(kt == kmax),
                    )
            ytrs = []
            for h in range(HPC):
                ytr = ytr_pool.tile([D + 1, QQ], BF16, name=f"ytr_{h}")
                _copy(ytr_engines[h], ytr, ps_ya[h])
                ytrs.append(ytr)
            return ytrs

        def emit_av_mm(b, avp, ytr_engines):
            return [emit_av_qc(b, qc, avp, ytr_engines) for qc in range(T // QQ)]

        def emit_norm_tail_qc(b, qc, ytrs, avt, do_tails=True, alternate=False):
            # alternate=True flips the whole per-q-tile chain (normalize +
            # evicts) between DVE and ScalarE so the two engines' in-order
            # FIFOs pipeline adjacent q-tiles instead of one FIFO carrying
            # every step of every chain.
            for qt in range(QQ // P):
                gq = b * KT + qc * (QQ // P) + qt
                eng = "scalar" if (alternate and qt % 2) else "vector"
                for h in range(HPC):
                    ps_t = avt.tile([P, D + 1], BF16, name="av_t")
                    nc.tensor.transpose(
                        ps_t,
                        ytrs[h][:, qt * P : (qt + 1) * P],
                        ident[0 : D + 1, 0 : D + 1],
                    )
                    rc = recips.tile([P, 1], FP32, name=f"rc_{h}")
                    nc.vector.reciprocal(rc, ps_t[:, D : D + 1])
                    yn_slice = yn_all[:, gq * P + h * D : gq * P + (h + 1) * D]
                    if eng == "scalar":
                        nc.scalar.mul(yn_slice, ps_t[:, 0:D], rc)
                    else:
                        nc.vector.tensor_scalar_mul(
                            out=yn_slice, in0=ps_t[:, 0:D], scalar1=rc
                        )
                if do_tails:
                    emit_tail(gq, o_engines=(eng, eng), yt_engine=eng)

        def emit_av_norm_tail(b, ytrs_all, avt, do_tails=True, alternate=False):
            for qc in range(T // QQ):
                emit_norm_tail_qc(b, qc, ytrs_all[qc], avt, do_tails, alternate)

        # b=0 tokens live in x chunks 0..T//XC-1
        emit_qkv0()
        qps0.release()
        # s_ps_0 sits at the bottom of the PSUM stack so qkv/vt (released
        # earlier) free their banks for the AV pools while S(b0) is live.
        # The pool is REUSED for S(b1): b1's window tiles WAR-chase b0's
        # drain, so exp(b1) starts while AV(b0) is still consuming pt.
        sps0 = tc.alloc_tile_pool(name="s_ps_0", bufs=1, space="PSUM")
        qps = tc.alloc_tile_pool(name="qkv_ps", bufs=2, space="PSUM")
        # S(b0) window 0 only needs chunk 0, so the exp stream starts ~12us
        # in; the remaining windows follow qkv(1) (their q-columns span
        # chunk 1 -- Tile cannot order reads against later-emitted writes).
        mask0 = set()
        emit_s(0, sps0, 0, 1, mask0)
        # chunk 1 chases chunk 0's pieces on the gpsimd queue (no bandwidth
        # competition with them); chunks 2-3 ride sync after w+bias.
        emit_qkv(1, q="gpsimd")
        nc.sync.dma_start(out=w_p_sb, in_=w_p)
        with tc.tile_pool(name="vt_ps", bufs=2, space="PSUM") as vtp:
            emit_v(0, vtp)
            emit_s(0, sps0, 1, PT_COLS // NQ, mask0)
            for nch in range(T // XC, TOK // XC):
                emit_qkv(nch)
            emit_v(1, vtp)
        qps.release()
        xp.release()
        with tc.tile_pool(name="av_ps_0", bufs=1, space="PSUM") as avp, \
             tc.tile_pool(name="avt_ps_0", bufs=2, space="PSUM") as avt:
            # ytr evicts split scalar/vector: ScalarE idles between the two
            # exp streams here.
            y0 = emit_av_mm(0, avp, ("scalar", "vector"))
            emit_av_norm_tail(0, y0, avt, do_tails=False)
        # b0's S PSUM is drained; b1 gets a fresh 2-bank pool (nq=512) so
        # the projection/transpose/AV pools can coexist with it: the whole
        # back half (S(b1) -> exp -> AV -> norm -> proj -> store) runs as
        # one dataflow region instead of three serial pool blocks.
        sps0.release()
        with tc.tile_pool(name="s_ps_1", bufs=1, space="PSUM") as sps1, \
             tc.tile_pool(name="o_ps", bufs=2, space="PSUM") as ops, \
             tc.tile_pool(name="yt_ps", bufs=1, space="PSUM") as tps2, \
             tc.tile_pool(name="av_ps_1", bufs=1, space="PSUM") as avp1, \
             tc.tile_pool(name="avt_ps_1", bufs=1, space="PSUM") as avt:
            # b0 projection tails FIRST in the PE stream: the PE is
            # saturated either way, but this makes their DVE evicts
            # runnable while exp(b1) owns ScalarE -- DVE would otherwise
            # idle through S(b1) and bunch all chains after it.
            for gq in range(KT):
                emit_tail(gq)
            emit_s(1, sps1, nq=512)
            # b1: norm+tail chains ride right behind each AV chunk; exp is
            # done once AV starts, so chains alternate DVE/ScalarE.
            for qc in range(T // QQ):
                ytrs = emit_av_qc(1, qc, avp1, ("scalar", "vector"))
                emit_norm_tail_qc(1, qc, ytrs, avt, alternate=True)


def shard_inputs(x, W_attn, b_attn, W_proj, b_proj):
    x = np.asarray(x, np.float32)
    W_attn = np.asarray(W_attn, np.float32)
    b_attn = np.asarray(b_attn, np.float32)
    W_proj = np.asarray(W_proj, np.float32)

    # [chunk, p, ktile, tok]: contiguous per-chunk DMA source for x^T
    xT = np.ascontiguousarray(
        x.reshape(TOK // XC, XC, C // P, P).transpose(0, 3, 2, 1)
    ).astype(bf16)
    in_maps = []
    for c in range(N_CORES):
        fs = slice(P * c, P * (c + 1))
        w_slice = np.ascontiguousarray(
            np.concatenate(
                [W_attn[:, 0 * C + P * c : 0 * C + P * (c + 1)],
                 W_attn[:, 1 * C + P * c : 1 * C + P * (c + 1)],
                 W_attn[:, 2 * C + P * c : 2 * C + P * (c + 1)]],
                axis=1,
            )
        ).astype(bf16)
        # [128, 3] host-prepack: column m = bias of feature m*128+p, so the
        # on-core DMA is 128 contiguous 12B lines instead of 384 4B gathers.
        b_slice = np.ascontiguousarray(
            np.stack([b_attn[0 * C + P * c : 0 * C + P * (c + 1)],
                      b_attn[1 * C + P * c : 1 * C + P * (c + 1)],
                      b_attn[2 * C + P * c : 2 * C + P * (c + 1)]], axis=1)
        ).astype(np.float32)
        wp_slice = np.ascontiguousarray(W_proj[fs, :]).astype(bf16)
        in_maps.append(
            {"xT": xT, "w_qkv": w_slice, "b_qkv": b_slice, "w_p": wp_slice}
        )
    return in_maps


def kernel(x, W_attn, b_attn, W_proj, b_proj, _trace=False):
    in_maps = shard_inputs(x, W_attn, b_attn, W_proj, b_proj)
    nc = build_nc()
    res = run_bass_kernel_spmd(nc, in_maps, list(range(N_CORES)), trace=_trace)
    acc = np.zeros((TOK, C), np.float64)
    for r in res.results:
        acc += r["out_p"].astype(np.float64)
    out = acc.astype(np.float32) + np.asarray(b_proj, np.float32)[None, :]
    if _trace:
        kernel.last_results = res
    return out.reshape(B, T, C)

